# revision 1
# baseline (speedup 1.0000x reference)
"""GAT attention layer (nn_AttentionLayer) on 8 Trainium2 NeuronCores.

Row-sharded outputs: core c owns output rows I_c = [c*N/8, (c+1)*N/8).
Inputs are laid out transposed on the host (same values, column-major
shards — a sharding/layout choice): each core receives
    adjT  = adj[I_c, :].T          [N, N/8]   int32
    featT = features.T             [D, N]     f32   (replicated)
    featT_loc = features[I_c].T    [D, N/8]   f32
so the device needs NO transposes, NO collectives — one pure stream:

    h = X@W, s1 = h@a1, s2 = h@a2 computed redundantly per core in fp16
    (PE matmuls over DMA-cast fp16 X^T tiles).
    Per 512-row j-quad (j on partitions, local i on the free axis):
        m  = adjT*BIG + (s2_j - BIG)     (DVE tensor_scalar, per j-tile)
        x  = m + s1_bcast                (DVE tensor_tensor)
        t  = 0.2x; y = max(x, t)         (DVE; leaky_relu)
        P^T = exp(y - 4)                 (ACT fp16; masked lanes -> exact 0)
        pso[it] += P^T_slice.T @ [h|s1|s2|1]   (PE fp16, fp32 accum)
    out = elu(pso[:, :F] * rcp(pso[:, F+2]))
"""

import os
import sys

for _p in ("/opt/trn_rl_repo",):
    if os.path.isdir(_p) and _p not in sys.path:
        sys.path.append(_p)

import numpy as np

import concourse.bass as bass
import concourse.bacc as bacc
import concourse.mybir as mybir
import concourse.tile as tile
from concourse import bass_utils

N, D, F = 8192, 256, 64
NCORES = 8
RL = N // NCORES
BIG = 240.0
ALPHA = 0.2
CSHIFT = 4.0

f32 = mybir.dt.float32
fp16 = mybir.dt.float16
i32 = mybir.dt.int32
Alu = mybir.AluOpType
Act = mybir.ActivationFunctionType

LAST_RESULTS = None
_CACHE = {}


def _kernel_body(tc, out_d, featT_d, featTl_d, adjT_d, W_d, a_d, n=N, rl=RL):
    nc = tc.nc
    nit = rl // 128           # local i-tiles
    njt = n // 128            # global j-tiles
    nk = D // 128             # d contraction tiles
    QT = 4                    # j-tiles per chain quad
    NQ = njt // QT
    HC = F + 3                # rhs cols: h(64) | s1 | s2 | ones
    NXC = 4                   # X^T streamed in chunks along j
    jxc = n // NXC

    s1d = nc.dram_tensor("s1bounce", [rl], fp16, kind="Internal").ap()

    with (
        tc.tile_pool(name="sbP", bufs=1) as sbP,
        tc.tile_pool(name="sbS", bufs=2) as sbS,
        tc.tile_pool(name="sbA", bufs=8) as sbA,
        tc.tile_pool(name="sbW", bufs=6) as sbW,
        tc.tile_pool(name="sbE", bufs=4) as sbE,
        tc.tile_pool(name="pp", bufs=8, space="PSUM") as pp,
    ):
        # ---- SWDGE ring: local X^T, then full X^T chunks, then adjT -------
        xTl = sbP.tile([128, nk, rl], fp16)
        nc.gpsimd.dma_start(xTl[:], featTl_d.rearrange("(k p) i -> p k i", p=128))

        xTf = [
            sbS.tile([128, nk, jxc], fp16, tag="xTf", name=f"xTf{i}")
            for i in range(NXC)
        ]
        ftr = featT_d.rearrange("(k p) (c j) -> c p k j", p=128, c=NXC)
        nc.gpsimd.dma_start(xTf[0][:], ftr[0])
        nc.gpsimd.dma_start(xTf[1][:], ftr[1])

        aq = [
            sbA.tile([128, QT, rl], fp16, tag="aq", name=f"aq{q}") for q in range(NQ)
        ]
        aqr = adjT_d.rearrange("(Q t p) i -> Q p t i", t=QT, p=128)

        def adj_load(q):
            nc.gpsimd.dma_start(aq[q][:], aqr[q])

        for q0 in range(min(2, NQ)):
            adj_load(q0)
        nc.gpsimd.dma_start(xTf[2][:], ftr[2])
        nc.gpsimd.dma_start(xTf[3][:], ftr[3])
        for q0 in range(2, min(4, NQ)):
            adj_load(q0)

        # ---- constants ----------------------------------------------------
        cshift = sbP.tile([128, 1], f32)
        nc.vector.memset(cshift[:], -CSHIFT)
        arow = sbP.tile([1, 2 * F], f32)
        nc.sync.dma_start(arow[:], a_d.rearrange("f o -> o f"))
        onesf = sbP.tile([1, 128], f32)
        nc.vector.memset(onesf[:], 1.0)
        ab = sbP.tile([128, 2 * F], f32)
        psab = pp.tile([128, 2 * F], f32, tag="big", name="psab")
        nc.tensor.matmul(psab[:], onesf[:], arow[:])
        nc.vector.tensor_copy(ab[:], psab[:])
        wsb = sbP.tile([128, nk, F], f32)
        nc.sync.dma_start(wsb[:], W_d.rearrange("(k p) f -> p k f", p=128))
        wa = sbP.tile([128, nk, 2], f32)
        scr = sbP.tile([128, F], f32)
        for k in range(nk):
            nc.vector.scalar_tensor_tensor(
                scr[:], wsb[:, k, :], 1.0, ab[:, :F], Alu.mult, Alu.mult,
                accum_out=wa[:, k, 0:1],
            )
            nc.vector.scalar_tensor_tensor(
                scr[:], wsb[:, k, :], 1.0, ab[:, F:], Alu.mult, Alu.mult,
                accum_out=wa[:, k, 1:2],
            )
        rhs16 = sbP.tile([128, nk, F + 2], fp16)
        for k in range(nk):
            nc.vector.tensor_copy(rhs16[:, k, :F], wsb[:, k, :])
            nc.vector.tensor_copy(rhs16[:, k, F : F + 2], wa[:, k, :])

        # ---- s1 local -> DRAM bounce -> free-axis broadcast tile ----------
        s1c16 = sbP.tile([128, nit], fp16)
        for it in range(nit):
            ps1 = pp.tile([128, 1], f32, tag="big", name=f"ps1_{it}")
            for k in range(nk):
                nc.tensor.matmul(
                    ps1[:], xTl[:, k, it * 128 : (it + 1) * 128], rhs16[:, k, F : F + 1],
                    start=(k == 0), stop=(k == nk - 1),
                )
            nc.vector.tensor_copy(s1c16[:, it : it + 1], ps1[:])
        nc.sync.dma_start(s1d.rearrange("(t p) -> p t", p=128), s1c16[:])
        s1row = sbP.tile([1, rl], fp16)
        nc.sync.dma_start(s1row[:], s1d.rearrange("(o j) -> o j", o=1))
        ones1 = sbP.tile([1, 128], fp16)
        nc.vector.memset(ones1[:], 1.0)
        s1b4 = sbP.tile([128, QT, rl], fp16)
        for cc0 in range(0, rl, 512):
            wch = min(512, rl - cc0)
            psb = pp.tile([128, wch], f32, tag="big", name=f"psb{cc0}")
            nc.tensor.matmul(psb[:], ones1[:], s1row[:, cc0 : cc0 + wch])
            nc.vector.tensor_copy(s1b4[:, 0, cc0 : cc0 + wch], psb[:])
        for tt in range(1, QT):
            nc.vector.tensor_copy(s1b4[:, tt, :], s1b4[:, 0, :])

        # ---- [h|s1|s2] for all rows (fp16 matmuls over streamed X^T) ------
        hs_all = sbP.tile([128, njt, HC], fp16)
        nc.vector.memset(hs_all[:, :, F + 2 : F + 3], 1.0)
        for cx in range(NXC):
            for tt in range(jxc // 128):
                t = cx * (jxc // 128) + tt
                psh = pp.tile([128, F + 2], f32, tag="big", name=f"psh{t}")
                for k in range(nk):
                    nc.tensor.matmul(
                        psh[:], xTf[cx][:, k, tt * 128 : (tt + 1) * 128],
                        rhs16[:, k, :],
                        start=(k == 0), stop=(k == nk - 1),
                    )
                nc.scalar.copy(hs_all[:, t, : F + 2], psh[:])

        # s2 - BIG, per-partition scalars per j-tile (fp32)
        s2mB = sbP.tile([128, njt], f32)

        # ---- attention chains over j-quads --------------------------------
        work = [
            sbW.tile([128, QT, rl], fp16, tag="work", name=f"work{q}") for q in range(NQ)
        ]
        lt = [
            sbW.tile([128, QT * rl], fp16, tag="lt", name=f"lt{q}", bufs=2)
            for q in range(NQ)
        ]
        pso = [
            pp.tile([128, HC], f32, tag="big", name=f"pso{i}") for i in range(nit)
        ]

        for q in range(NQ):
            if q + 4 < NQ:
                adj_load(q + 4)
            nc.vector.tensor_scalar(
                s2mB[:, q * QT : (q + 1) * QT],
                hs_all[:, q * QT : (q + 1) * QT, F + 1],
                -BIG, None, Alu.add,
            )
            w = work[q]
            for tt in range(QT):
                nc.vector.tensor_scalar(
                    w[:, tt, :], aq[q][:, tt, :], BIG,
                    s2mB[:, q * QT + tt : q * QT + tt + 1],
                    Alu.mult, Alu.add,
                )
            wf = w[:].rearrange("p t i -> p (t i)")
            s1f = s1b4[:].rearrange("p t i -> p (t i)")
            nc.vector.tensor_tensor(wf, wf, s1f, Alu.add)
            nc.vector.tensor_scalar_mul(lt[q][:], wf, ALPHA)
            nc.vector.tensor_tensor(wf, wf, lt[q][:], Alu.max)
            nc.scalar.activation(wf, wf, Act.Exp, bias=cshift[:], scale=1.0)
            for tt in range(QT):
                t = q * QT + tt
                for it in range(nit):
                    nc.tensor.matmul(
                        pso[it][:],
                        w[:, tt, it * 128 : (it + 1) * 128],
                        hs_all[:, t, :],
                        start=(t == 0), stop=(t == njt - 1),
                    )

        # ---- epilogue ------------------------------------------------------
        for it in range(nit):
            ps = pso[it]
            rcp = sbE.tile([128, 1], f32, tag="rcp")
            nc.vector.reciprocal(rcp[:], ps[:, F + 2 : F + 3])
            o = sbE.tile([128, F], f32, tag="o")
            nc.vector.tensor_scalar_mul(o[:], ps[:, :F], rcp[:])
            q2 = sbE.tile([128, F], f32, tag="q2")
            nc.vector.tensor_scalar_min(q2[:], o[:], 0.0)
            e = sbE.tile([128, F], f32, tag="e")
            nc.scalar.activation(e[:], q2[:], Act.Exp)
            r = sbE.tile([128, F], f32, tag="r")
            nc.vector.tensor_scalar_max(r[:], o[:], 0.0)
            fin = sbE.tile([128, F], f32, tag="fin")
            nc.vector.scalar_tensor_tensor(
                fin[:], e[:], -1.0, r[:], Alu.add, Alu.add
            )
            nc.sync.dma_start(out_d[it * 128 : (it + 1) * 128, :], fin[:])


def _build(n=N, rl=RL, ncores=NCORES):
    key = (n, rl, ncores)
    if key in _CACHE:
        return _CACHE[key]
    nc = bacc.Bacc(
        "TRN2", target_bir_lowering=False, debug=False, num_devices=ncores
    )
    featT = nc.dram_tensor("featT", [D, n], f32, kind="ExternalInput").ap()
    featTl = nc.dram_tensor("featTl", [D, rl], f32, kind="ExternalInput").ap()
    adjT = nc.dram_tensor("adjT", [n, rl], i32, kind="ExternalInput").ap()
    W = nc.dram_tensor("W", [D, F], f32, kind="ExternalInput").ap()
    a = nc.dram_tensor("a", [2 * F, 1], f32, kind="ExternalInput").ap()
    out = nc.dram_tensor("out", [rl, F], f32, kind="ExternalOutput").ap()
    with tile.TileContext(nc) as tc:
        _kernel_body(tc, out, featT, featTl, adjT, W, a, n=n, rl=rl)
    nc.compile()
    _CACHE[key] = nc
    return nc


def kernel(features, adj, W, a):
    global LAST_RESULTS
    features = np.ascontiguousarray(features, dtype=np.float32)
    adj = np.ascontiguousarray(adj, dtype=np.int32)
    W = np.ascontiguousarray(W, dtype=np.float32)
    a = np.ascontiguousarray(a, dtype=np.float32)

    n = adj.shape[0]
    rl = n // NCORES
    nc = _build(n=n, rl=rl, ncores=NCORES)
    featT = np.ascontiguousarray(features.T)
    in_maps = [
        {
            "featT": featT,
            "featTl": np.ascontiguousarray(features[c * rl : (c + 1) * rl].T),
            "adjT": np.ascontiguousarray(adj[c * rl : (c + 1) * rl].T),
            "W": W,
            "a": a,
        }
        for c in range(NCORES)
    ]
    res = bass_utils.run_bass_kernel_spmd(nc, in_maps, core_ids=list(range(NCORES)))
    LAST_RESULTS = res
    return np.concatenate([res.results[c]["out"] for c in range(NCORES)], axis=0)



# revision 2
# speedup vs baseline: 1.0745x; 1.0745x over previous
"""GAT attention layer (nn_AttentionLayer) on 8 Trainium2 NeuronCores.

Row-sharded outputs: core c owns output rows I_c = [c*N/8, (c+1)*N/8).
Host-side layout (same values, layout/precision staging only):
    adjT  = adj[I_c, :].T * 40    [N, N/8]  fp16 {0, 40}
    featT = features.T            [D, N]    fp16 (replicated)
    featT_loc = features[I_c].T   [D, N/8]  fp16
Device pipeline per 512-row j-quad (j on partitions, local i on free axis):
    x    = adjT40 + (s2_j - 40) + s1_i      (DVE STT, per j-tile scalar)
    y    = prelu(x, 0.2)                    (ACT parametric relu, one call)
    bits = int16(y*A + B)                   (DVE TS 4x; Schraudolph exp:
                                             bitcast fp16 ~= exp(y))
    pso[it] += bits.as_fp16.T @ [h|1]       (PE fp16, fp32 accum)
    out = elu(pso[:, :F] * rcp(pso[:, F]))
Masked entries: x ~= s1+s2-40 -> y ~= 0.2x -> exp(y) ~ e^-8, negligible
vs row mass; no explicit mask multiply or -inf needed.
"""

import os
import sys

for _p in ("/opt/trn_rl_repo",):
    if os.path.isdir(_p) and _p not in sys.path:
        sys.path.append(_p)

import numpy as np

import concourse.bass as bass
import concourse.bacc as bacc
import concourse.mybir as mybir
import concourse.tile as tile
from concourse import bass_utils

N, D, F = 8192, 256, 64
NCORES = 8
RL = N // NCORES
BIG = 40.0
ALPHA = 0.2
# Schraudolph fp16 exp: bitcast_f16(int16(A*y + B)) ~= e^y, A = 2^10/ln2,
# B = 15360 - 61 (61 centers the mantissa-linearization error).
SCH_A = 1477.3196
SCH_B = 15299.0

f32 = mybir.dt.float32
fp16 = mybir.dt.float16
i16 = mybir.dt.int16
Alu = mybir.AluOpType
Act = mybir.ActivationFunctionType

LAST_RESULTS = None
_CACHE = {}


def _kernel_body(tc, out_d, featT_d, featTl_d, adjT_d, W_d, a_d, n=N, rl=RL):
    nc = tc.nc
    nit = rl // 128           # local i-tiles
    njt = n // 128            # global j-tiles
    nk = D // 128             # d contraction tiles
    QT = 4                    # j-tiles per chain quad
    NQ = njt // QT
    HC = F + 3                # hs cols: h(64) | ones | s1 | s2
    AC = F + 1                # aggregated cols: h | ones
    NXC = 4                   # X^T streamed in chunks along j
    jxc = n // NXC

    s1d = nc.dram_tensor("s1bounce", [rl], fp16, kind="Internal").ap()

    with (
        tc.tile_pool(name="sbP", bufs=1) as sbP,
        tc.tile_pool(name="sbS", bufs=2) as sbS,
        tc.tile_pool(name="sbA", bufs=8) as sbA,
        tc.tile_pool(name="sbL", bufs=3) as sbL,
        tc.tile_pool(name="sbE", bufs=4) as sbE,
        tc.tile_pool(name="pp", bufs=8, space="PSUM") as pp,
    ):
        # ---- SWDGE ring: local X^T, then full X^T chunks, then adjT -------
        xTl = sbP.tile([128, nk, rl], fp16)
        nc.gpsimd.dma_start(xTl[:], featTl_d.rearrange("(k p) i -> p k i", p=128))

        xTf = [
            sbS.tile([128, nk, jxc], fp16, tag="xTf", name=f"xTf{i}")
            for i in range(NXC)
        ]
        ftr = featT_d.rearrange("(k p) (c j) -> c p k j", p=128, c=NXC)
        nc.gpsimd.dma_start(xTf[0][:], ftr[0])
        nc.gpsimd.dma_start(xTf[1][:], ftr[1])

        aq = [
            sbA.tile([128, QT, rl], fp16, tag="aq", name=f"aq{q}") for q in range(NQ)
        ]
        aqr = adjT_d.rearrange("(Q t p) i -> Q p t i", t=QT, p=128)

        def adj_load(q):
            nc.gpsimd.dma_start(aq[q][:], aqr[q])

        for q0 in range(min(2, NQ)):
            adj_load(q0)
        nc.gpsimd.dma_start(xTf[2][:], ftr[2])
        nc.gpsimd.dma_start(xTf[3][:], ftr[3])
        for q0 in range(2, min(4, NQ)):
            adj_load(q0)

        # ---- constants ----------------------------------------------------
        arow = sbP.tile([1, 2 * F], f32)
        nc.sync.dma_start(arow[:], a_d.rearrange("f o -> o f"))
        onesf = sbP.tile([1, 128], f32)
        nc.vector.memset(onesf[:], 1.0)
        ab = sbP.tile([128, 2 * F], f32)
        psab = pp.tile([128, 2 * F], f32, tag="big", name="psab")
        nc.tensor.matmul(psab[:], onesf[:], arow[:])
        nc.vector.tensor_copy(ab[:], psab[:])
        wsb = sbP.tile([128, nk, F], f32)
        nc.sync.dma_start(wsb[:], W_d.rearrange("(k p) f -> p k f", p=128))
        wa = sbP.tile([128, nk, 2], f32)
        scr = sbP.tile([128, F], f32)
        for k in range(nk):
            nc.vector.scalar_tensor_tensor(
                scr[:], wsb[:, k, :], 1.0, ab[:, :F], Alu.mult, Alu.mult,
                accum_out=wa[:, k, 0:1],
            )
            nc.vector.scalar_tensor_tensor(
                scr[:], wsb[:, k, :], 1.0, ab[:, F:], Alu.mult, Alu.mult,
                accum_out=wa[:, k, 1:2],
            )
        rhs16 = sbP.tile([128, nk, F + 2], fp16)
        for k in range(nk):
            nc.vector.tensor_copy(rhs16[:, k, :F], wsb[:, k, :])
            nc.vector.tensor_copy(rhs16[:, k, F : F + 2], wa[:, k, :])

        # ---- s1 local -> DRAM bounce -> free-axis broadcast tile ----------
        s1c16 = sbP.tile([128, nit], fp16)
        for it in range(nit):
            ps1 = pp.tile([128, 1], f32, tag="big", name=f"ps1_{it}")
            for k in range(nk):
                nc.tensor.matmul(
                    ps1[:], xTl[:, k, it * 128 : (it + 1) * 128], rhs16[:, k, F : F + 1],
                    start=(k == 0), stop=(k == nk - 1),
                )
            nc.vector.tensor_copy(s1c16[:, it : it + 1], ps1[:])
        nc.sync.dma_start(s1d.rearrange("(t p) -> p t", p=128), s1c16[:])
        s1row = sbP.tile([1, rl], fp16)
        nc.sync.dma_start(s1row[:], s1d.rearrange("(o j) -> o j", o=1))
        ones1 = sbP.tile([1, 128], fp16)
        nc.vector.memset(ones1[:], 1.0)
        s1b4 = sbP.tile([128, QT, rl], fp16)
        for cc0 in range(0, rl, 512):
            wch = min(512, rl - cc0)
            psb = pp.tile([128, wch], f32, tag="big", name=f"psb{cc0}")
            nc.tensor.matmul(psb[:], ones1[:], s1row[:, cc0 : cc0 + wch])
            nc.vector.tensor_copy(s1b4[:, 0, cc0 : cc0 + wch], psb[:])
        for tt in range(1, QT):
            nc.vector.tensor_copy(s1b4[:, tt, :], s1b4[:, 0, :])

        # ---- [h|1|s1|s2] for all rows (fp16 matmuls over streamed X^T) ----
        hs_all = sbP.tile([128, njt, HC], fp16)
        nc.vector.memset(hs_all[:, :, F : F + 1], 1.0)
        for cx in range(NXC):
            for tt in range(jxc // 128):
                t = cx * (jxc // 128) + tt
                psh = pp.tile([128, F + 2], f32, tag="big", name=f"psh{t}")
                for k in range(nk):
                    nc.tensor.matmul(
                        psh[:], xTf[cx][:, k, tt * 128 : (tt + 1) * 128],
                        rhs16[:, k, :],
                        start=(k == 0), stop=(k == nk - 1),
                    )
                nc.scalar.copy(hs_all[:, t, :F], psh[:, :F])
                nc.scalar.copy(hs_all[:, t, F + 1 : F + 3], psh[:, F : F + 2])

        # s2 - 40, per-partition scalars per j-tile (fp32)
        s2mB = sbP.tile([128, njt], f32)

        # ---- attention chains over j-quads --------------------------------
        lt = [
            sbL.tile([128, QT, rl], i16, tag="lt", name=f"lt{q}") for q in range(NQ)
        ]
        pso = [
            pp.tile([128, AC], f32, tag="big", name=f"pso{i}") for i in range(nit)
        ]

        for q in range(NQ):
            if q + 4 < NQ:
                adj_load(q + 4)
            nc.vector.tensor_scalar(
                s2mB[:, q * QT : (q + 1) * QT],
                hs_all[:, q * QT : (q + 1) * QT, F + 2],
                -BIG, None, Alu.add,
            )
            w = aq[q]
            for tt in range(QT):
                t = q * QT + tt
                nc.vector.scalar_tensor_tensor(
                    w[:, tt, :], w[:, tt, :], s2mB[:, t : t + 1], s1b4[:, tt, :],
                    Alu.add, Alu.add,
                )
            wf = w[:].rearrange("p t i -> p (t i)")
            nc.scalar.activation(wf, wf, Act.Prelu, alpha=ALPHA)
            ltf = lt[q][:].rearrange("p t i -> p (t i)")
            nc.vector.tensor_scalar(ltf, wf, SCH_A, SCH_B, Alu.mult, Alu.add)
            w16 = lt[q][:].bitcast(fp16)
            for tt in range(QT):
                t = q * QT + tt
                for it in range(nit):
                    nc.tensor.matmul(
                        pso[it][:],
                        w16[:, tt, it * 128 : (it + 1) * 128],
                        hs_all[:, t, :AC],
                        start=(t == 0), stop=(t == njt - 1),
                    )

        # ---- epilogue ------------------------------------------------------
        for it in range(nit):
            ps = pso[it]
            rcp = sbE.tile([128, 1], f32, tag="rcp")
            nc.vector.reciprocal(rcp[:], ps[:, F : F + 1])
            o = sbE.tile([128, F], f32, tag="o")
            nc.vector.tensor_scalar_mul(o[:], ps[:, :F], rcp[:])
            q2 = sbE.tile([128, F], f32, tag="q2")
            nc.vector.tensor_scalar_min(q2[:], o[:], 0.0)
            e = sbE.tile([128, F], f32, tag="e")
            nc.scalar.activation(e[:], q2[:], Act.Exp)
            r = sbE.tile([128, F], f32, tag="r")
            nc.vector.tensor_scalar_max(r[:], o[:], 0.0)
            fin = sbE.tile([128, F], f32, tag="fin")
            nc.vector.scalar_tensor_tensor(
                fin[:], e[:], -1.0, r[:], Alu.add, Alu.add
            )
            nc.sync.dma_start(out_d[it * 128 : (it + 1) * 128, :], fin[:])


def _build(n=N, rl=RL, ncores=NCORES):
    key = (n, rl, ncores)
    if key in _CACHE:
        return _CACHE[key]
    nc = bacc.Bacc(
        "TRN2", target_bir_lowering=False, debug=False, num_devices=ncores
    )
    featT = nc.dram_tensor("featT", [D, n], fp16, kind="ExternalInput").ap()
    featTl = nc.dram_tensor("featTl", [D, rl], fp16, kind="ExternalInput").ap()
    adjT = nc.dram_tensor("adjT", [n, rl], fp16, kind="ExternalInput").ap()
    W = nc.dram_tensor("W", [D, F], f32, kind="ExternalInput").ap()
    a = nc.dram_tensor("a", [2 * F, 1], f32, kind="ExternalInput").ap()
    out = nc.dram_tensor("out", [rl, F], f32, kind="ExternalOutput").ap()
    with tile.TileContext(nc) as tc:
        _kernel_body(tc, out, featT, featTl, adjT, W, a, n=n, rl=rl)
    nc.compile()
    _CACHE[key] = nc
    return nc


def kernel(features, adj, W, a):
    global LAST_RESULTS
    features = np.ascontiguousarray(features, dtype=np.float32)
    adj = np.ascontiguousarray(adj, dtype=np.int32)
    W = np.ascontiguousarray(W, dtype=np.float32)
    a = np.ascontiguousarray(a, dtype=np.float32)

    n = adj.shape[0]
    rl = n // NCORES
    nc = _build(n=n, rl=rl, ncores=NCORES)
    featT = np.ascontiguousarray(features.T.astype(np.float16))
    in_maps = [
        {
            "featT": featT,
            "featTl": np.ascontiguousarray(
                features[c * rl : (c + 1) * rl].T.astype(np.float16)
            ),
            "adjT": np.ascontiguousarray(
                adj[c * rl : (c + 1) * rl].T.astype(np.float16) * np.float16(BIG)
            ),
            "W": W,
            "a": a,
        }
        for c in range(NCORES)
    ]
    res = bass_utils.run_bass_kernel_spmd(nc, in_maps, core_ids=list(range(NCORES)))
    LAST_RESULTS = res
    return np.concatenate([res.results[c]["out"] for c in range(NCORES)], axis=0)


# revision 11
# speedup vs baseline: 1.3726x; 1.2775x over previous
"""GAT attention layer (nn_AttentionLayer) on 8 Trainium2 NeuronCores.

Row-sharded outputs: core c owns output rows I_c = [c*N/8, (c+1)*N/8).
Host-side layout (same values, layout/precision staging only):
    adjT  = adj[I_c, :].T * 40    [N, N/8]  fp16 {0, 40}
    featT = features.T            [D, N]    fp16 (replicated)
    featT_loc = features[I_c].T   [D, N/8]  fp16
Device pipeline per 512-row j-quad (j on partitions, local i on free axis):
    x1   = adjT40 + s1_i                    (DVE TT, 2x)
    y    = prelu(x1 + (s2_j-40), 0.2)       (ACT parametric relu, bias/tile)
    bits = int16(y*A + B)                   (DVE TS 4x; Schraudolph exp:
                                             bitcast fp16 ~= exp(y))
    agg[j-tile] : psum[c,i] += hs[j,c].T @ bits.as_fp16[j,i]   (hs as
                  PE weights, scores streamed; 64 big matmuls)
    out = elu(num/den) after a PE transpose of the [67, rl] accumulator.
Masked entries: x ~= s1+s2-40 -> y ~= 0.2x -> exp(y) ~ e^-8, negligible
vs row mass; no explicit mask multiply or -inf needed.
"""

import os
import sys

for _p in ("/opt/trn_rl_repo",):
    if os.path.isdir(_p) and _p not in sys.path:
        sys.path.append(_p)

import numpy as np

import concourse.bass as bass
import concourse.bacc as bacc
import concourse.mybir as mybir
import concourse.tile as tile
from concourse import bass_utils

N, D, F = 8192, 256, 64
NCORES = 8
RL = N // NCORES
BIG = 40.0
ALPHA = 0.2
# Schraudolph fp16 exp: bitcast_f16(int16(A*y + B)) ~= e^y, A = 2^10/ln2,
# B = 15360 - 61 (61 centers the mantissa-linearization error).
SCH_A = 1477.3196
SCH_B = 15299.0

f32 = mybir.dt.float32
fp16 = mybir.dt.float16
i16 = mybir.dt.int16
Alu = mybir.AluOpType
Act = mybir.ActivationFunctionType

LAST_RESULTS = None
_CACHE = {}


def _kernel_body(tc, out_d, featT_d, featTl_d, adjT_d, W_d, a_d, idn_d, n=N, rl=RL):
    nc = tc.nc
    nit = rl // 128           # local i-tiles
    njt = n // 128            # global j-tiles
    nk = D // 128             # d contraction tiles
    QT = 4                    # j-tiles per chain quad
    NQ = njt // QT
    HC = F + 3                # hs cols: h(64) | s1 | s2 | ones
    NXC = 4                   # X^T streamed in chunks along j
    jxc = n // NXC

    s1d = nc.dram_tensor("s1bounce", [rl], fp16, kind="Internal").ap()

    with (
        tc.tile_pool(name="sbP", bufs=1) as sbP,
        tc.tile_pool(name="sbS", bufs=2) as sbS,
        tc.tile_pool(name="sbA", bufs=8) as sbA,
        tc.tile_pool(name="sbL", bufs=3) as sbL,
        tc.tile_pool(name="sbE", bufs=4) as sbE,
        tc.tile_pool(name="pp", bufs=6, space="PSUM") as pp,
        tc.tile_pool(name="pacc", bufs=1, space="PSUM") as pacc,
    ):
        # ---- DMA rings: local X^T, then full X^T chunks, then adjT --------
        # adj quads alternate gpsimd/sync queues for DMA parallelism.
        xTl = sbP.tile([128, nk, rl], fp16)
        nc.gpsimd.dma_start(xTl[:], featTl_d.rearrange("(k p) i -> p k i", p=128))

        xTf = [
            sbS.tile([128, nk, jxc], fp16, tag="xTf", name=f"xTf{i}")
            for i in range(NXC)
        ]
        ftr = featT_d.rearrange("(k p) (c j) -> c p k j", p=128, c=NXC)
        nc.gpsimd.dma_start(xTf[0][:], ftr[0])
        nc.sync.dma_start(xTf[1][:], ftr[1])

        aq = [
            sbA.tile([128, QT, rl], fp16, tag="aq", name=f"aq{q}") for q in range(NQ)
        ]
        aqr = adjT_d.rearrange("(Q t p) i -> Q p t i", t=QT, p=128)

        def adj_load(q):
            eng = nc.gpsimd if (q % 2 == 0) else nc.sync
            eng.dma_start(aq[q][:], aqr[q])

        for q0 in range(min(2, NQ)):
            adj_load(q0)
        nc.gpsimd.dma_start(xTf[2][:], ftr[2])
        nc.sync.dma_start(xTf[3][:], ftr[3])
        for q0 in range(2, min(4, NQ)):
            adj_load(q0)

        # ---- constants ----------------------------------------------------
        arow = sbP.tile([1, 2 * F], f32)
        nc.sync.dma_start(arow[:], a_d.rearrange("f o -> o f"))
        onesf = sbP.tile([1, 128], f32)
        nc.vector.memset(onesf[:], 1.0)
        ab = sbP.tile([128, 2 * F], f32)
        psab = pp.tile([128, 2 * F], f32, tag="big", name="psab")
        nc.tensor.matmul(psab[:], onesf[:], arow[:])
        nc.vector.tensor_copy(ab[:], psab[:])
        wsb = sbP.tile([128, nk, F], f32)
        nc.sync.dma_start(wsb[:], W_d.rearrange("(k p) f -> p k f", p=128))
        wa = sbP.tile([128, nk, 2], f32)
        scr = sbP.tile([128, F], f32)
        for k in range(nk):
            nc.vector.scalar_tensor_tensor(
                scr[:], wsb[:, k, :], 1.0, ab[:, :F], Alu.mult, Alu.mult,
                accum_out=wa[:, k, 0:1],
            )
            nc.vector.scalar_tensor_tensor(
                scr[:], wsb[:, k, :], 1.0, ab[:, F:], Alu.mult, Alu.mult,
                accum_out=wa[:, k, 1:2],
            )
        rhs16 = sbP.tile([128, nk, F + 2], fp16)
        for k in range(nk):
            nc.vector.tensor_copy(rhs16[:, k, :F], wsb[:, k, :])
            nc.vector.tensor_copy(rhs16[:, k, F : F + 2], wa[:, k, :])

        # ---- s1 local -> DRAM bounce -> free-axis broadcast tile ----------
        s1c16 = sbP.tile([128, nit], fp16)
        for it in range(nit):
            ps1 = pp.tile([128, 1], f32, tag="big", name=f"ps1_{it}")
            for k in range(nk):
                nc.tensor.matmul(
                    ps1[:], xTl[:, k, it * 128 : (it + 1) * 128], rhs16[:, k, F : F + 1],
                    start=(k == 0), stop=(k == nk - 1),
                )
            nc.vector.tensor_copy(s1c16[:, it : it + 1], ps1[:])
        nc.sync.dma_start(s1d.rearrange("(t p) -> p t", p=128), s1c16[:])
        s1row = sbP.tile([1, rl], fp16)
        nc.sync.dma_start(s1row[:], s1d.rearrange("(o j) -> o j", o=1))
        ones1 = sbP.tile([1, 128], fp16)
        nc.vector.memset(ones1[:], 1.0)
        s1b4 = sbP.tile([128, QT, rl], fp16)
        for cc0 in range(0, rl, 512):
            wch = min(512, rl - cc0)
            psb = pp.tile([128, wch], f32, tag="big", name=f"psb{cc0}")
            nc.tensor.matmul(psb[:], ones1[:], s1row[:, cc0 : cc0 + wch])
            nc.vector.tensor_copy(s1b4[:, 0, cc0 : cc0 + wch], psb[:])
        for tt in range(1, QT):
            nc.vector.tensor_copy(s1b4[:, tt, :], s1b4[:, 0, :])

        # ---- [h|s1|s2|1] for all rows (batched psum, one copy per quad) ---
        hs_all = sbP.tile([128, njt, HC], fp16)
        nc.vector.memset(hs_all[:, :, F + 2 : F + 3], 1.0)
        for g in range(njt // QT):
            psh = pp.tile([128, QT, F + 2], f32, tag="big", name=f"psh{g}")
            for tt in range(QT):
                t = g * QT + tt
                cx, ct = divmod(t, jxc // 128)
                for k in range(nk):
                    nc.tensor.matmul(
                        psh[:, tt, :], xTf[cx][:, k, ct * 128 : (ct + 1) * 128],
                        rhs16[:, k, :],
                        start=(k == 0), stop=(k == nk - 1),
                    )
            nc.scalar.copy(hs_all[:, g * QT : (g + 1) * QT, : F + 2], psh[:])

        # s2 - 40, per-partition scalars per j-tile (fp32)
        s2mB = sbP.tile([128, njt], f32)

        # ---- attention chains over j-quads --------------------------------
        lt = [
            sbL.tile([128, QT, rl], i16, tag="lt", name=f"lt{q}") for q in range(NQ)
        ]
        # hs-as-weights accumulator: [HC, rl] += hs[:,t,:].T @ P[:, i]
        pso = pacc.tile([HC, rl], f32, name="pso")

        for q in range(NQ):
            if q + 4 < NQ:
                adj_load(q + 4)
            nc.vector.tensor_scalar(
                s2mB[:, q * QT : (q + 1) * QT],
                hs_all[:, q * QT : (q + 1) * QT, F + 1],
                -BIG, None, Alu.add,
            )
            w = aq[q]
            wf = w[:].rearrange("p t i -> p (t i)")
            s1f = s1b4[:].rearrange("p t i -> p (t i)")
            nc.vector.tensor_tensor(wf, wf, s1f, Alu.add)
            for tt in range(QT):
                t = q * QT + tt
                nc.scalar.activation(
                    w[:, tt, :], w[:, tt, :], Act.Prelu,
                    bias=s2mB[:, t : t + 1], alpha=ALPHA,
                )
            ltf = lt[q][:].rearrange("p t i -> p (t i)")
            nc.vector.tensor_scalar(ltf, wf, SCH_A, SCH_B, Alu.mult, Alu.add)
            w16 = lt[q][:].bitcast(fp16)
            for tt in range(QT):
                t = q * QT + tt
                for hh in range(0, rl, 512):
                    nc.tensor.matmul(
                        pso[:, hh : hh + 512], hs_all[:, t, :],
                        w16[:, tt, hh : hh + 512],
                        start=(t == 0), stop=(t == njt - 1),
                    )

        # ---- epilogue: PE-transpose pso chunks, divide, elu ---------------
        psof = sbE.tile([HC, rl], f32, tag="psof", bufs=1)
        nc.vector.tensor_copy(psof[:], pso[:])
        idn = sbE.tile([HC, HC], f32, tag="idn", bufs=1)
        nc.sync.dma_start(idn[:], idn_d)
        for it in range(nit):
            pst = pp.tile([128, HC], f32, tag="big", name=f"pst{it}")
            nc.tensor.transpose(
                pst[:, :], psof[:, it * 128 : (it + 1) * 128], idn[:]
            )
            rcp = sbE.tile([128, 1], f32, tag="rcp")
            nc.vector.reciprocal(rcp[:], pst[:, F + 2 : F + 3])
            o = sbE.tile([128, F], f32, tag="o")
            nc.vector.tensor_scalar_mul(o[:], pst[:, :F], rcp[:])
            q2 = sbE.tile([128, F], f32, tag="q2")
            nc.vector.tensor_scalar_min(q2[:], o[:], 0.0)
            e = sbE.tile([128, F], f32, tag="e")
            nc.scalar.activation(e[:], q2[:], Act.Exp)
            r = sbE.tile([128, F], f32, tag="r")
            nc.vector.tensor_scalar_max(r[:], o[:], 0.0)
            fin = sbE.tile([128, F], f32, tag="fin")
            nc.vector.scalar_tensor_tensor(
                fin[:], e[:], -1.0, r[:], Alu.add, Alu.add
            )
            nc.sync.dma_start(out_d[it * 128 : (it + 1) * 128, :], fin[:])


def _build(n=N, rl=RL, ncores=NCORES):
    key = (n, rl, ncores)
    if key in _CACHE:
        return _CACHE[key]
    nc = bacc.Bacc(
        "TRN2", target_bir_lowering=False, debug=False, num_devices=ncores
    )
    featT = nc.dram_tensor("featT", [D, n], fp16, kind="ExternalInput").ap()
    featTl = nc.dram_tensor("featTl", [D, rl], fp16, kind="ExternalInput").ap()
    adjT = nc.dram_tensor("adjT", [n, rl], fp16, kind="ExternalInput").ap()
    W = nc.dram_tensor("W", [D, F], f32, kind="ExternalInput").ap()
    a = nc.dram_tensor("a", [2 * F, 1], f32, kind="ExternalInput").ap()
    idn = nc.dram_tensor("idn", [F + 3, F + 3], f32, kind="ExternalInput").ap()
    out = nc.dram_tensor("out", [rl, F], f32, kind="ExternalOutput").ap()
    with tile.TileContext(nc) as tc:
        _kernel_body(tc, out, featT, featTl, adjT, W, a, idn, n=n, rl=rl)
    nc.compile()
    _CACHE[key] = nc
    return nc


def kernel(features, adj, W, a):
    global LAST_RESULTS
    features = np.ascontiguousarray(features, dtype=np.float32)
    adj = np.ascontiguousarray(adj, dtype=np.int32)
    W = np.ascontiguousarray(W, dtype=np.float32)
    a = np.ascontiguousarray(a, dtype=np.float32)

    n = adj.shape[0]
    rl = n // NCORES
    nc = _build(n=n, rl=rl, ncores=NCORES)
    featT = np.ascontiguousarray(features.T.astype(np.float16))
    idn = np.eye(F + 3, dtype=np.float32)
    in_maps = [
        {
            "featT": featT,
            "featTl": np.ascontiguousarray(
                features[c * rl : (c + 1) * rl].T.astype(np.float16)
            ),
            "adjT": np.ascontiguousarray(
                adj[c * rl : (c + 1) * rl].T.astype(np.float16) * np.float16(BIG)
            ),
            "W": W,
            "a": a,
            "idn": idn,
        }
        for c in range(NCORES)
    ]
    res = bass_utils.run_bass_kernel_spmd(nc, in_maps, core_ids=list(range(NCORES)))
    LAST_RESULTS = res
    return np.concatenate([res.results[c]["out"] for c in range(NCORES)], axis=0)
